# revision 16
# baseline (speedup 1.0000x reference)
"""BiAttention Trainium2 kernel (nn_BiAttention_76794015252634).

reference math (mode=1), per batch b:
    proj_h = attn @ Wh.T + bh          # [Wn, D]
    scores = main @ proj_h.T           # [T, Wn]
    probs  = softmax(scores, axis=-1)
    out_h  = probs @ attn              # [T, D]
for h in {2, 3}; returns (out_2, out_3).

Design notes (v3):
  * bias bh cancels in softmax (row-constant) -> skipped.
  * softmax shift: fixed constant C=100 instead of per-row max; scores are
    built transposed (w-major) so no transposes of the probabilities.
  * denominator Z falls out of the context matmul via a ones-column in attn.
  * D (scores) stationary operand projT is fp16: its LDWEIGHTS gets fast
    weight load and hides fully under the previous matmul's 213ns fill,
    unlike fp32's 190ns exposed load. The moving operand stays fp32r.
  * the 44-row tail of the D=300 contraction is fp16 and ROW-PACKED via PE
    tile rows 0:64 / 64:128 so the two tail matmuls of a wc-pair run
    concurrently; the pair leads the accumulation group.
  * scoresT slabs for a wc-pair land in one 2-bank PSUM tile and leave via a
    single 1024-wide exp; one of the 4 per-slab divides runs on the scalar
    engine so neither ACT nor DVE exceeds the PE's per-slab time.
  * outputs accumulate per (batch, head) in SBUF fp16 and leave per-slab in
    per-partition-contiguous DMAs on the sync queue; host undoes the
    [P, chunk] interleave and upcasts.
  * inputs arrive as consolidated per-partition-contiguous images (weights
    1 DMA/head, attn 1 DMA/batch with the k44 dup + zero pad baked in on
    host) spread across 4 engine queues; dummy warmup matmuls keep the PE
    HAM clock-gate warming while they land.

Per (batch, head):
    A: projT[d, w]   = sum_k WhT[k, d] attnT[k, w]          (PE, PSUM->SBUF)
    D: scoresT[w, t] = sum_d projT[d, w] mainT[d, t]        (PE)
       es[w, t]      = exp(scoresT - C)                     (ACT, PSUM->SBUF)
    F: [out | Z][t]  = sum_w es[w, t] [attn | 1][w, :]      (PE)
       out[t, d]     = out[t, d] / Z[t]                     (DVE recip + mul)

Sharding: data-parallel over batch, B=16 -> 2 batches per core on 8 cores.
"""

import ml_dtypes
import numpy as np

import concourse.bass as bass
import concourse.tile as tile
from concourse import bacc, mybir
from concourse import bass_utils

B, T, Wn, D = 16, 2048, 512, 300
NCORES = 8
BPC = B // NCORES  # batches per core
P = 128
WCH = Wn // P      # 4 w-chunks
TS = 512           # t slab width (one PSUM bank)
TSN = T // TS      # 4 slabs
NCH = T // P       # 16 output t-chunks per (b,h)
K44 = 44           # tail rows of the D=300 contraction
CBIAS = 100.0      # softmax shift constant (see module docstring)
NWU = 8            # warmup matmuls

F32 = mybir.dt.float32
F32R = mybir.dt.float32r
BF16 = mybir.dt.bfloat16
F16 = mybir.dt.float16

_cached = None


def _build_program():
    nc = bacc.Bacc("TRN2", target_bir_lowering=False, debug=False)

    mainT = nc.dram_tensor("mainT", [BPC, 256, T], F16, kind="ExternalInput").ap()
    attnTr = nc.dram_tensor("attnTr", [BPC, P, 4, Wn], F16, kind="ExternalInput").ap()
    attnF = nc.dram_tensor("attnF", [BPC, P, WCH, D + 2], BF16, kind="ExternalInput").ap()
    wTr = nc.dram_tensor("wTr", [P, 2, 4, D], F16, kind="ExternalInput").ap()
    main44 = nc.dram_tensor("main44", [BPC, K44, T], F16, kind="ExternalInput").ap()
    outs = [
        nc.dram_tensor(f"out{h}", [BPC, P, NCH, D], F16, kind="ExternalOutput").ap()
        for h in range(2)
    ]

    Exp = mybir.ActivationFunctionType.Exp
    Copy = mybir.ActivationFunctionType.Copy

    with tile.TileContext(nc) as tc:
        with (
            tc.tile_pool(name="consts", bufs=1) as consts,
            tc.tile_pool(name="batch", bufs=2) as batch_pool,
            tc.tile_pool(name="proj", bufs=2) as proj_pool,
            tc.tile_pool(name="work", bufs=2) as work,
            tc.tile_pool(name="outp", bufs=2) as outp,
            tc.tile_pool(name="stats", bufs=8) as stats,
            tc.tile_pool(name="pd", bufs=3, space="PSUM") as pd,    # 1 tag x 2 banks
            tc.tile_pool(name="pf", bufs=1, space="PSUM") as pf,    # 1 tag x 2 banks
        ):
            # PE warmup scratch first (gpsimd boots early)
            wz = consts.tile([P, TS], BF16, tag="wz")
            nc.gpsimd.memset(wz[:], 0.0)
            nbias = consts.tile([P, 1], F32, tag="nbias")
            nc.vector.memset(nbias[:], -CBIAS)

            wu = pd.tile([P, 2, Wn], F32, name="wu", tag="ps_dm")
            for i in range(NWU):
                nc.tensor.matmul(
                    wu[:, 0, :], wz[:, :P], wz[:], start=(i == 0), stop=(i == NWU - 1)
                )

            # projection weights, transposed: wt_sb[h][k % 128, kc, d] = W_h[d, k]
            # (host bakes in: chunk2 = rows 256:300 at partitions 0:44, chunk3 =
            # the same rows duplicated at partitions 64:108, zeros elsewhere)
            wt_sb = []
            for h in range(2):
                t_ = consts.tile([P, 4, D], F16, tag=f"wt{h}")
                nc.sync.dma_start(t_[:], wTr[:, h, :, :])
                wt_sb.append(t_)

            # --- input loads for ALL batches up front, spread across queues,
            # critical-path first; later transfers overlap compute ---
            HT = T // 2
            ats, mains, m44s, afs = [], [], [], []
            # attnTr for ALL batches first: the scheduler hoists later
            # batches' A-phase matmuls into the first batch's pipeline, so
            # their attn data must land early or the PE FIFO head-of-line
            # blocks on it
            for b in range(BPC):
                at_sb = batch_pool.tile([P, 4, Wn], F16, tag="attnT")
                nc.scalar.dma_start(at_sb[:, 0:2, :], attnTr[b, :, 0:2, :])
                nc.gpsimd.dma_start(at_sb[:, 2:4, :], attnTr[b, :, 2:4, :])
                ats.append(at_sb)
            for b in range(BPC):
                m44b = batch_pool.tile([P, T], F16, tag="m44")
                nc.gpsimd.dma_start(m44b[:K44, :], main44[b])
                nc.gpsimd.dma_start(m44b[64 : 64 + K44, :], main44[b])
                # main halves: kc0 on gpsimd, kc1 on sync
                main_sb = batch_pool.tile([P, 2, T], F16, tag="main")
                af_sb = batch_pool.tile([P, WCH, D + 2], BF16, tag="attnF")
                nc.gpsimd.dma_start(main_sb[:, 0, 0:HT], mainT[b, 0:P, 0:HT])
                nc.sync.dma_start(main_sb[:, 1, 0:HT], mainT[b, P : 2 * P, 0:HT])
                nc.gpsimd.dma_start(af_sb[:], attnF[b])
                nc.gpsimd.dma_start(main_sb[:, 0, HT:T], mainT[b, 0:P, HT:T])
                nc.sync.dma_start(main_sb[:, 1, HT:T], mainT[b, P : 2 * P, HT:T])
                mains.append(main_sb)
                m44s.append(m44b)
                afs.append(af_sb)

            for b in range(BPC):
                at_sb, main_sb, m44b, af_sb = ats[b], mains[b], m44s[b], afs[b]

                # --- A: projections for BOTH heads up front.  h=0 uses the pa
                # banks, h=1 borrows a pd ring slot (D hasn't started yet) so
                # h=1's matmuls never wait on h=0's PSUM evacuation. ---
                projTs, projT44s = [], []
                for h in range(2):
                    projT = proj_pool.tile([P, 2, Wn], F16, tag=f"projT{h}")
                    projT44b = proj_pool.tile([P, Wn], F16, tag=f"projT44{h}")
                    pam = pd.tile([P, 2, Wn], F32, name="pam", tag="ps_dm")
                    for kc in (0, 1):
                        for j in range(2):
                            nc.tensor.matmul(
                                pam[:, j, :],
                                wt_sb[h][:P, kc, j * P : (j + 1) * P],
                                at_sb[:P, kc, :],
                                start=(kc == 0),
                                stop=False,
                            )
                    # k44 tail, row-packed: j=0 in rows 0:64, j=1 in rows 64:128
                    nc.tensor.matmul(
                        pam[:, 0, :],
                        wt_sb[h][:K44, 2, 0:P],
                        at_sb[:K44, 2, :],
                        start=False,
                        stop=True,
                    )
                    nc.tensor.matmul(
                        pam[:, 1, :],
                        wt_sb[h][64 : 64 + K44, 3, P : 2 * P],
                        at_sb[64 : 64 + K44, 3, :],
                        start=False,
                        stop=True,
                    )
                    nc.scalar.activation(projT[:], pam[:], Copy)
                    # proj rows 256:300 -> fp16 straight from PSUM (pf bank is
                    # idle during the A phase)
                    pfm2 = pf.tile([P, 2, Wn], F32, name="ps_a2", tag="ps_fm")
                    pa2 = pfm2[:, 0, :]
                    for kc in (0, 1):
                        nc.tensor.matmul(
                            pa2[:K44, :],
                            wt_sb[h][:P, kc, 2 * P : D],
                            at_sb[:P, kc, :],
                            start=(kc == 0),
                            stop=False,
                        )
                    nc.tensor.matmul(
                        pa2[:K44, :],
                        wt_sb[h][:K44, 2, 2 * P : D],
                        at_sb[:K44, 2, :],
                        start=False,
                        stop=True,
                    )
                    nc.scalar.activation(projT44b[:K44, :], pa2[:K44, :], Copy)
                    # duplicate at partitions 64:108 for row-packed D tails
                    nc.scalar.dma_start(projT44b[64 : 64 + K44, :], projT44b[:K44, :])
                    projTs.append(projT)
                    projT44s.append(projT44b)

                for h in range(2):
                    projT, projT44b = projTs[h], projT44s[h]
                    o_big = outp.tile([P, NCH, D], F16, tag=f"o{h}")
                    for t5 in range(TSN):
                        ts0 = t5 * TS
                        # D: scoresT[w, t] for a wc-pair into a 2-bank PSUM
                        # tile; packed k44 pair leads, then fp32r chunks.
                        es = work.tile([P, WCH, TS], BF16, tag="es")
                        for wp in range(WCH // 2):
                            wcs = (2 * wp, 2 * wp + 1)
                            pdm = pd.tile([P, 2, TS], F32, name="ps_dm", tag="ps_dm")
                            for kc in (0, 1):
                                for j, wc in enumerate(wcs):
                                    nc.tensor.matmul(
                                        pdm[:, j, :],
                                        projT[:P, kc, wc * P : (wc + 1) * P],
                                        main_sb[:P, kc, ts0 : ts0 + TS],
                                        start=(kc == 0),
                                        stop=False,
                                    )
                            # k44 tails, row-packed -> concurrent pair
                            nc.tensor.matmul(
                                pdm[:, 0, :],
                                projT44b[:K44, wcs[0] * P : (wcs[0] + 1) * P],
                                m44b[:K44, ts0 : ts0 + TS],
                                start=False,
                                stop=True,
                            )
                            nc.tensor.matmul(
                                pdm[:, 1, :],
                                projT44b[64 : 64 + K44, wcs[1] * P : (wcs[1] + 1) * P],
                                m44b[64 : 64 + K44, ts0 : ts0 + TS],
                                start=False,
                                stop=True,
                            )
                            nc.scalar.activation(
                                es[:, 2 * wp : 2 * wp + 2, :],
                                pdm[:],
                                Exp,
                                bias=nbias[:],
                                scale=1.0,
                            )
                        # F: [out | Z] = es.T @ [attn | 1]; out /= Z
                        for tp in range(TS // P // 2):
                            tcs = (2 * tp * P, (2 * tp + 1) * P)
                            pfm = pf.tile([P, 2, Wn], F32, name="ps_fm", tag="ps_fm")
                            pfs = [pfm[:, 0, :], pfm[:, 1, :]]
                            for wc in range(WCH):
                                for j, tc0 in enumerate(tcs):
                                    nc.tensor.matmul(
                                        pfs[j][:, : D + 2],
                                        es[:, wc, tc0 : tc0 + P],
                                        af_sb[:, wc, :],
                                        start=(wc == 0),
                                        stop=(wc == WCH - 1),
                                    )
                            last = b == BPC - 1 and h == 1 and t5 == TSN - 1
                            for j, tc0 in enumerate(tcs):
                                c = t5 * (TS // P) + tc0 // P
                                rz = stats.tile([P, 1], F32, tag="rz")
                                nc.vector.reciprocal(rz[:], pfs[j][:, D : D + 1])
                                if last and tp == 0:
                                    # final slab: split divides across ACT and
                                    # DVE so they finish concurrently
                                    nc.scalar.activation(
                                        o_big[:, c, :], pfs[j][:, :D], Copy, scale=rz[:]
                                    )
                                else:
                                    nc.vector.tensor_scalar_mul(
                                        o_big[:, c, :], pfs[j][:, :D], rz[:]
                                    )
                            if last:
                                for j, tc0 in enumerate(tcs):
                                    c = t5 * (TS // P) + tc0 // P
                                    nc.gpsimd.dma_start(
                                        outs[h][b, :, c : c + 1, :], o_big[:, c : c + 1, :]
                                    )
                        if not (b == BPC - 1 and h == 1 and t5 == TSN - 1):
                            c0 = t5 * (TS // P)
                            nc.gpsimd.dma_start(
                                outs[h][b, :, c0 : c0 + 4, :], o_big[:, c0 : c0 + 4, :]
                            )

    nc.compile()
    return nc


def _get_program():
    global _cached
    if _cached is None:
        _cached = _build_program()
    return _cached


def _prep_in_maps(input1, input2, W2, W3):
    input1 = np.ascontiguousarray(input1, dtype=np.float32)
    input2 = np.ascontiguousarray(input2, dtype=np.float32)
    wt = np.stack([np.asarray(W2).T, np.asarray(W3).T]).astype(np.float32)  # [2, D, D]
    wtr = np.zeros((P, 2, 4, D), np.float32)
    for kc in (0, 1):
        wtr[:, :, kc, :] = wt[:, kc * P : (kc + 1) * P, :].transpose(1, 0, 2)
    wtr[:K44, :, 2, :] = wt[:, 2 * P : D, :].transpose(1, 0, 2)
    wtr[64 : 64 + K44, :, 3, :] = wt[:, 2 * P : D, :].transpose(1, 0, 2)
    wtr = np.ascontiguousarray(wtr).astype(np.float16)
    in_maps = []
    for c in range(NCORES):
        sl = slice(c * BPC, (c + 1) * BPC)
        i1t = input1[sl].transpose(0, 2, 1)  # [BPC, D, T]
        i2 = input2[sl]
        i2t = i2.transpose(0, 2, 1)  # [BPC, D, Wn]
        atr = np.zeros((BPC, P, 4, Wn), np.float32)
        for kc in (0, 1):
            atr[:, :, kc, :] = i2t[:, kc * P : (kc + 1) * P, :]
        atr[:, :K44, 2, :] = i2t[:, 2 * P : D, :]
        atr[:, 64 : 64 + K44, 3, :] = i2t[:, 2 * P : D, :]
        af = np.ones((BPC, WCH, P, D + 2), np.float32)
        af[:, :, :, :D] = i2.reshape(BPC, WCH, P, D)
        in_maps.append(
            {
                "mainT": np.ascontiguousarray(i1t[:, :256, :]).astype(np.float16),
                "attnTr": np.ascontiguousarray(atr).astype(np.float16),
                "attnF": np.ascontiguousarray(af.transpose(0, 2, 1, 3)).astype(ml_dtypes.bfloat16),
                "wTr": wtr,
                "main44": np.ascontiguousarray(i1t[:, 256:D, :]).astype(np.float16),
            }
        )
    return in_maps


def kernel(input1, input2, W2, b2, W3, b3, mode, _trace=False):
    mode = int(np.asarray(mode))
    if mode not in (0, 1):
        raise AttributeError("Wrong mode!")

    nc = _get_program()
    in_maps = _prep_in_maps(input1, input2, W2, W3)
    res = bass_utils.run_bass_kernel_spmd(
        nc, in_maps, core_ids=list(range(NCORES)), trace=_trace
    )
    full_outs = []
    for h in range(2):
        parts = [np.asarray(r[f"out{h}"]) for r in res.results]  # [BPC, P, NCH, D] f16
        full = np.concatenate(parts, axis=0)  # [B, P, NCH, D]
        full = (
            full.transpose(0, 2, 1, 3).reshape(B, T, D).astype(np.float32)
        )  # t = c*128 + p
        full_outs.append(full)
    if _trace:
        kernel.last_results = res
    if mode == 0:
        return full_outs[0]
    return (full_outs[0], full_outs[1])


# revision 17
# speedup vs baseline: 1.1430x; 1.1430x over previous
"""BiAttention Trainium2 kernel (nn_BiAttention_76794015252634).

reference math (mode=1), per batch b:
    proj_h = attn @ Wh.T + bh          # [Wn, D]
    scores = main @ proj_h.T           # [T, Wn]
    probs  = softmax(scores, axis=-1)
    out_h  = probs @ attn              # [T, D]
for h in {2, 3}; returns (out_2, out_3).

Design notes (v3):
  * bias bh cancels in softmax (row-constant) -> skipped.
  * softmax shift: fixed constant C=100 instead of per-row max; scores are
    built transposed (w-major) so no transposes of the probabilities.
  * denominator Z falls out of the context matmul via a ones-column in attn.
  * D (scores) stationary operand projT is fp16: its LDWEIGHTS gets fast
    weight load and hides fully under the previous matmul's 213ns fill,
    unlike fp32's 190ns exposed load. The moving operand stays fp32r.
  * the 44-row tail of the D=300 contraction is fp16 and ROW-PACKED via PE
    tile rows 0:64 / 64:128 so the two tail matmuls of a wc-pair run
    concurrently; the pair leads the accumulation group.
  * scoresT slabs for a wc-pair land in one 2-bank PSUM tile and leave via a
    single 1024-wide exp; one of the 4 per-slab divides runs on the scalar
    engine so neither ACT nor DVE exceeds the PE's per-slab time.
  * outputs accumulate per (batch, head) in SBUF fp16 and leave per-slab in
    per-partition-contiguous DMAs on the sync queue; host undoes the
    [P, chunk] interleave and upcasts.
  * inputs arrive as consolidated per-partition-contiguous images (weights
    1 DMA/head, attn 1 DMA/batch with the k44 dup + zero pad baked in on
    host) spread across 4 engine queues; dummy warmup matmuls keep the PE
    HAM clock-gate warming while they land.

Per (batch, head):
    A: projT[d, w]   = sum_k WhT[k, d] attnT[k, w]          (PE, PSUM->SBUF)
    D: scoresT[w, t] = sum_d projT[d, w] mainT[d, t]        (PE)
       es[w, t]      = exp(scoresT - C)                     (ACT, PSUM->SBUF)
    F: [out | Z][t]  = sum_w es[w, t] [attn | 1][w, :]      (PE)
       out[t, d]     = out[t, d] / Z[t]                     (DVE recip + mul)

Sharding: data-parallel over batch, B=16 -> 2 batches per core on 8 cores.
"""

import ml_dtypes
import numpy as np

import concourse.bass as bass
import concourse.tile as tile
from concourse import bacc, mybir
from concourse import bass_utils

B, T, Wn, D = 16, 2048, 512, 300
NCORES = 8
BPC = B // NCORES  # batches per core
P = 128
WCH = Wn // P      # 4 w-chunks
TS = 512           # t slab width (one PSUM bank)
TSN = T // TS      # 4 slabs
NCH = T // P       # 16 output t-chunks per (b,h)
K44 = 44           # tail rows of the D=300 contraction
CBIAS = 100.0      # softmax shift constant (see module docstring)
NWU = 8            # warmup matmuls

F32 = mybir.dt.float32
F32R = mybir.dt.float32r
BF16 = mybir.dt.bfloat16
F16 = mybir.dt.float16

_cached = None


def _build_program():
    nc = bacc.Bacc("TRN2", target_bir_lowering=False, debug=False)

    mainT = nc.dram_tensor("mainT", [BPC, 256, T], F16, kind="ExternalInput").ap()
    attnTr = nc.dram_tensor("attnTr", [BPC, P, 4, Wn], F16, kind="ExternalInput").ap()
    attnF = nc.dram_tensor("attnF", [BPC, P, WCH, D + 2], BF16, kind="ExternalInput").ap()
    wTr = nc.dram_tensor("wTr", [P, 2, 4, D], F16, kind="ExternalInput").ap()
    main44 = nc.dram_tensor("main44", [BPC, K44, T], F16, kind="ExternalInput").ap()
    outs = [
        nc.dram_tensor(f"out{h}", [BPC, P, NCH, D], F16, kind="ExternalOutput").ap()
        for h in range(2)
    ]

    Exp = mybir.ActivationFunctionType.Exp
    Copy = mybir.ActivationFunctionType.Copy

    with tile.TileContext(nc) as tc:
        with (
            tc.tile_pool(name="consts", bufs=1) as consts,
            tc.tile_pool(name="batch", bufs=2) as batch_pool,
            tc.tile_pool(name="proj", bufs=2) as proj_pool,
            tc.tile_pool(name="work", bufs=2) as work,
            tc.tile_pool(name="outp", bufs=2) as outp,
            tc.tile_pool(name="stats", bufs=8) as stats,
            tc.tile_pool(name="pa", bufs=1, space="PSUM") as pa,    # 1 tag x 2 banks
            tc.tile_pool(name="pd", bufs=2, space="PSUM") as pd,    # 1 tag x 2 banks
            tc.tile_pool(name="pf", bufs=1, space="PSUM") as pf,    # 2 tags x 1 bank
        ):
            # PE warmup scratch first (gpsimd boots early)
            wz = consts.tile([P, TS], BF16, tag="wz")
            nc.gpsimd.memset(wz[:], 0.0)
            nbias = consts.tile([P, 1], F32, tag="nbias")
            nc.vector.memset(nbias[:], -CBIAS)

            wu = pa.tile([P, 2, Wn], F32, name="wu", tag="pam")
            for i in range(NWU):
                nc.tensor.matmul(
                    wu[:, 0, :], wz[:, :P], wz[:], start=(i == 0), stop=(i == NWU - 1)
                )

            # projection weights, transposed: wt_sb[h][k % 128, kc, d] = W_h[d, k]
            # (host bakes in: chunk2 = rows 256:300 at partitions 0:44, chunk3 =
            # the same rows duplicated at partitions 64:108, zeros elsewhere)
            wt_sb = []
            for h in range(2):
                t_ = consts.tile([P, 4, D], F16, tag=f"wt{h}")
                nc.sync.dma_start(t_[:], wTr[:, h, :, :])
                wt_sb.append(t_)

            # --- input loads for ALL batches up front, spread across queues,
            # critical-path first; later transfers overlap compute ---
            HT = T // 2
            ats, mains, m44s, afs = [], [], [], []
            # attnTr for ALL batches first: the scheduler hoists later
            # batches' A-phase matmuls into the first batch's pipeline, so
            # their attn data must land early or the PE FIFO head-of-line
            # blocks on it
            for b in range(BPC):
                at_sb = batch_pool.tile([P, 4, Wn], F16, tag="attnT")
                nc.scalar.dma_start(at_sb[:, 0:2, :], attnTr[b, :, 0:2, :])
                nc.gpsimd.dma_start(at_sb[:, 2:4, :], attnTr[b, :, 2:4, :])
                ats.append(at_sb)
            for b in range(BPC):
                m44b = batch_pool.tile([P, T], F16, tag="m44")
                nc.gpsimd.dma_start(m44b[:K44, :], main44[b])
                nc.gpsimd.dma_start(m44b[64 : 64 + K44, :], main44[b])
                # main halves: kc0 on gpsimd, kc1 on sync
                main_sb = batch_pool.tile([P, 2, T], F16, tag="main")
                af_sb = batch_pool.tile([P, WCH, D + 2], BF16, tag="attnF")
                nc.gpsimd.dma_start(main_sb[:, 0, 0:HT], mainT[b, 0:P, 0:HT])
                nc.sync.dma_start(main_sb[:, 1, 0:HT], mainT[b, P : 2 * P, 0:HT])
                nc.gpsimd.dma_start(af_sb[:], attnF[b])
                nc.gpsimd.dma_start(main_sb[:, 0, HT:T], mainT[b, 0:P, HT:T])
                nc.sync.dma_start(main_sb[:, 1, HT:T], mainT[b, P : 2 * P, HT:T])
                mains.append(main_sb)
                m44s.append(m44b)
                afs.append(af_sb)

            for b in range(BPC):
                at_sb, main_sb, m44b, af_sb = ats[b], mains[b], m44s[b], afs[b]

                # --- A: projections for BOTH heads up front.  h=0 uses the pa
                # banks, h=1 borrows a pd ring slot (D hasn't started yet) so
                # h=1's matmuls never wait on h=0's PSUM evacuation. ---
                projTs, projT44s = [], []
                for h in range(2):
                    projT = proj_pool.tile([P, 2, Wn], F16, tag=f"projT{h}")
                    projT44b = proj_pool.tile([P, Wn], F16, tag=f"projT44{h}")
                    if h == 0:
                        pam = pa.tile([P, 2, Wn], F32, name="pam", tag="pam")
                    else:
                        pam = pd.tile([P, 2, Wn], F32, name="pam_d", tag="ps_dm")
                    for kc in (0, 1):
                        for j in range(2):
                            nc.tensor.matmul(
                                pam[:, j, :],
                                wt_sb[h][:P, kc, j * P : (j + 1) * P],
                                at_sb[:P, kc, :],
                                start=(kc == 0),
                                stop=False,
                            )
                    # k44 tail, row-packed: j=0 in rows 0:64, j=1 in rows 64:128
                    nc.tensor.matmul(
                        pam[:, 0, :],
                        wt_sb[h][:K44, 2, 0:P],
                        at_sb[:K44, 2, :],
                        start=False,
                        stop=True,
                    )
                    nc.tensor.matmul(
                        pam[:, 1, :],
                        wt_sb[h][64 : 64 + K44, 3, P : 2 * P],
                        at_sb[64 : 64 + K44, 3, :],
                        start=False,
                        stop=True,
                    )
                    nc.scalar.activation(projT[:], pam[:], Copy)
                    # proj rows 256:300 -> fp16 straight from PSUM (pf bank is
                    # idle during the A phase)
                    pa2 = pf.tile([P, Wn], F32, name="ps_a2", tag=f"ps_f{h}")
                    for kc in (0, 1):
                        nc.tensor.matmul(
                            pa2[:K44, :],
                            wt_sb[h][:P, kc, 2 * P : D],
                            at_sb[:P, kc, :],
                            start=(kc == 0),
                            stop=False,
                        )
                    nc.tensor.matmul(
                        pa2[:K44, :],
                        wt_sb[h][:K44, 2, 2 * P : D],
                        at_sb[:K44, 2, :],
                        start=False,
                        stop=True,
                    )
                    nc.scalar.activation(projT44b[:K44, :], pa2[:K44, :], Copy)
                    # duplicate at partitions 64:108 for row-packed D tails
                    nc.scalar.dma_start(projT44b[64 : 64 + K44, :], projT44b[:K44, :])
                    projTs.append(projT)
                    projT44s.append(projT44b)

                for h in range(2):
                    projT, projT44b = projTs[h], projT44s[h]
                    o_big = outp.tile([P, NCH, D], F16, tag=f"o{h}")
                    for t5 in range(TSN):
                        ts0 = t5 * TS
                        # D: scoresT[w, t] for a wc-pair into a 2-bank PSUM
                        # tile; packed k44 pair leads, then fp32r chunks.
                        es = work.tile([P, WCH, TS], BF16, tag="es")
                        for wp in range(WCH // 2):
                            wcs = (2 * wp, 2 * wp + 1)
                            pdm = pd.tile([P, 2, TS], F32, name="ps_dm", tag="ps_dm")
                            for kc in (0, 1):
                                for j, wc in enumerate(wcs):
                                    nc.tensor.matmul(
                                        pdm[:, j, :],
                                        projT[:P, kc, wc * P : (wc + 1) * P],
                                        main_sb[:P, kc, ts0 : ts0 + TS],
                                        start=(kc == 0),
                                        stop=False,
                                    )
                            # k44 tails, row-packed -> concurrent pair
                            nc.tensor.matmul(
                                pdm[:, 0, :],
                                projT44b[:K44, wcs[0] * P : (wcs[0] + 1) * P],
                                m44b[:K44, ts0 : ts0 + TS],
                                start=False,
                                stop=True,
                            )
                            nc.tensor.matmul(
                                pdm[:, 1, :],
                                projT44b[64 : 64 + K44, wcs[1] * P : (wcs[1] + 1) * P],
                                m44b[64 : 64 + K44, ts0 : ts0 + TS],
                                start=False,
                                stop=True,
                            )
                            nc.scalar.activation(
                                es[:, 2 * wp : 2 * wp + 2, :],
                                pdm[:],
                                Exp,
                                bias=nbias[:],
                                scale=1.0,
                            )
                        # F: [out | Z] = es.T @ [attn | 1]; out /= Z
                        for tp in range(TS // P // 2):
                            tcs = (2 * tp * P, (2 * tp + 1) * P)
                            pfs = [
                                pf.tile([P, Wn], F32, name=f"ps_f{j}", tag=f"ps_f{j}")
                                for j in range(2)
                            ]
                            for wc in range(WCH):
                                for j, tc0 in enumerate(tcs):
                                    nc.tensor.matmul(
                                        pfs[j][:, : D + 2],
                                        es[:, wc, tc0 : tc0 + P],
                                        af_sb[:, wc, :],
                                        start=(wc == 0),
                                        stop=(wc == WCH - 1),
                                    )
                            last = b == BPC - 1 and h == 1 and t5 == TSN - 1
                            for j, tc0 in enumerate(tcs):
                                c = t5 * (TS // P) + tc0 // P
                                rz = stats.tile([P, 1], F32, tag="rz")
                                nc.vector.reciprocal(rz[:], pfs[j][:, D : D + 1])
                                if last and tp == 0:
                                    # final slab: split divides across ACT and
                                    # DVE so they finish concurrently
                                    nc.scalar.activation(
                                        o_big[:, c, :], pfs[j][:, :D], Copy, scale=rz[:]
                                    )
                                else:
                                    nc.vector.tensor_scalar_mul(
                                        o_big[:, c, :], pfs[j][:, :D], rz[:]
                                    )
                            if last:
                                for j, tc0 in enumerate(tcs):
                                    c = t5 * (TS // P) + tc0 // P
                                    nc.sync.dma_start(
                                        outs[h][b, :, c : c + 1, :], o_big[:, c : c + 1, :]
                                    )
                        if not (b == BPC - 1 and h == 1 and t5 == TSN - 1):
                            c0 = t5 * (TS // P)
                            nc.sync.dma_start(
                                outs[h][b, :, c0 : c0 + 4, :], o_big[:, c0 : c0 + 4, :]
                            )

    nc.compile()
    return nc


def _get_program():
    global _cached
    if _cached is None:
        _cached = _build_program()
    return _cached


def _prep_in_maps(input1, input2, W2, W3):
    input1 = np.ascontiguousarray(input1, dtype=np.float32)
    input2 = np.ascontiguousarray(input2, dtype=np.float32)
    wt = np.stack([np.asarray(W2).T, np.asarray(W3).T]).astype(np.float32)  # [2, D, D]
    wtr = np.zeros((P, 2, 4, D), np.float32)
    for kc in (0, 1):
        wtr[:, :, kc, :] = wt[:, kc * P : (kc + 1) * P, :].transpose(1, 0, 2)
    wtr[:K44, :, 2, :] = wt[:, 2 * P : D, :].transpose(1, 0, 2)
    wtr[64 : 64 + K44, :, 3, :] = wt[:, 2 * P : D, :].transpose(1, 0, 2)
    wtr = np.ascontiguousarray(wtr).astype(np.float16)
    in_maps = []
    for c in range(NCORES):
        sl = slice(c * BPC, (c + 1) * BPC)
        i1t = input1[sl].transpose(0, 2, 1)  # [BPC, D, T]
        i2 = input2[sl]
        i2t = i2.transpose(0, 2, 1)  # [BPC, D, Wn]
        atr = np.zeros((BPC, P, 4, Wn), np.float32)
        for kc in (0, 1):
            atr[:, :, kc, :] = i2t[:, kc * P : (kc + 1) * P, :]
        atr[:, :K44, 2, :] = i2t[:, 2 * P : D, :]
        atr[:, 64 : 64 + K44, 3, :] = i2t[:, 2 * P : D, :]
        af = np.ones((BPC, WCH, P, D + 2), np.float32)
        af[:, :, :, :D] = i2.reshape(BPC, WCH, P, D)
        in_maps.append(
            {
                "mainT": np.ascontiguousarray(i1t[:, :256, :]).astype(np.float16),
                "attnTr": np.ascontiguousarray(atr).astype(np.float16),
                "attnF": np.ascontiguousarray(af.transpose(0, 2, 1, 3)).astype(ml_dtypes.bfloat16),
                "wTr": wtr,
                "main44": np.ascontiguousarray(i1t[:, 256:D, :]).astype(np.float16),
            }
        )
    return in_maps


def kernel(input1, input2, W2, b2, W3, b3, mode, _trace=False):
    mode = int(np.asarray(mode))
    if mode not in (0, 1):
        raise AttributeError("Wrong mode!")

    nc = _get_program()
    in_maps = _prep_in_maps(input1, input2, W2, W3)
    res = bass_utils.run_bass_kernel_spmd(
        nc, in_maps, core_ids=list(range(NCORES)), trace=_trace
    )
    full_outs = []
    for h in range(2):
        parts = [np.asarray(r[f"out{h}"]) for r in res.results]  # [BPC, P, NCH, D] f16
        full = np.concatenate(parts, axis=0)  # [B, P, NCH, D]
        full = (
            full.transpose(0, 2, 1, 3).reshape(B, T, D).astype(np.float32)
        )  # t = c*128 + p
        full_outs.append(full)
    if _trace:
        kernel.last_results = res
    if mode == 0:
        return full_outs[0]
    return (full_outs[0], full_outs[1])


# revision 18
# speedup vs baseline: 1.1532x; 1.0089x over previous
"""BiAttention Trainium2 kernel (nn_BiAttention_76794015252634).

reference math (mode=1), per batch b:
    proj_h = attn @ Wh.T + bh          # [Wn, D]
    scores = main @ proj_h.T           # [T, Wn]
    probs  = softmax(scores, axis=-1)
    out_h  = probs @ attn              # [T, D]
for h in {2, 3}; returns (out_2, out_3).

Design notes (v3):
  * bias bh cancels in softmax (row-constant) -> skipped.
  * softmax shift: fixed constant C=100 instead of per-row max; scores are
    built transposed (w-major) so no transposes of the probabilities.
  * denominator Z falls out of the context matmul via a ones-column in attn.
  * D (scores) stationary operand projT is fp16: its LDWEIGHTS gets fast
    weight load and hides fully under the previous matmul's 213ns fill,
    unlike fp32's 190ns exposed load. The moving operand stays fp32r.
  * the 44-row tail of the D=300 contraction is fp16 and ROW-PACKED via PE
    tile rows 0:64 / 64:128 so the two tail matmuls of a wc-pair run
    concurrently; the pair leads the accumulation group.
  * scoresT slabs for a wc-pair land in one 2-bank PSUM tile and leave via a
    single 1024-wide exp; one of the 4 per-slab divides runs on the scalar
    engine so neither ACT nor DVE exceeds the PE's per-slab time.
  * outputs accumulate per (batch, head) in SBUF fp16 and leave per-slab in
    per-partition-contiguous DMAs on the sync queue; host undoes the
    [P, chunk] interleave and upcasts.
  * inputs arrive as consolidated per-partition-contiguous images (weights
    1 DMA/head, attn 1 DMA/batch with the k44 dup + zero pad baked in on
    host) spread across 4 engine queues; dummy warmup matmuls keep the PE
    HAM clock-gate warming while they land.

Per (batch, head):
    A: projT[d, w]   = sum_k WhT[k, d] attnT[k, w]          (PE, PSUM->SBUF)
    D: scoresT[w, t] = sum_d projT[d, w] mainT[d, t]        (PE)
       es[w, t]      = exp(scoresT - C)                     (ACT, PSUM->SBUF)
    F: [out | Z][t]  = sum_w es[w, t] [attn | 1][w, :]      (PE)
       out[t, d]     = out[t, d] / Z[t]                     (DVE recip + mul)

Sharding: data-parallel over batch, B=16 -> 2 batches per core on 8 cores.
"""

import ml_dtypes
import numpy as np

import concourse.bass as bass
import concourse.tile as tile
from concourse import bacc, mybir
from concourse import bass_utils

B, T, Wn, D = 16, 2048, 512, 300
NCORES = 8
BPC = B // NCORES  # batches per core
P = 128
WCH = Wn // P      # 4 w-chunks
TS = 512           # t slab width (one PSUM bank)
TSN = T // TS      # 4 slabs
NCH = T // P       # 16 output t-chunks per (b,h)
K44 = 44           # tail rows of the D=300 contraction
CBIAS = 100.0      # softmax shift constant (see module docstring)
NWU = 8            # warmup matmuls

F32 = mybir.dt.float32
F32R = mybir.dt.float32r
BF16 = mybir.dt.bfloat16
F16 = mybir.dt.float16

_cached = None


def _build_program():
    nc = bacc.Bacc("TRN2", target_bir_lowering=False, debug=False)

    mainT = nc.dram_tensor("mainT", [BPC, 256, T], F16, kind="ExternalInput").ap()
    attnTr = nc.dram_tensor("attnTr", [BPC, P, 4, Wn], F16, kind="ExternalInput").ap()
    attnF = nc.dram_tensor("attnF", [BPC, P, WCH, D + 2], BF16, kind="ExternalInput").ap()
    wTr = nc.dram_tensor("wTr", [P, 2, 4, D], F16, kind="ExternalInput").ap()
    main44 = nc.dram_tensor("main44", [BPC, K44, T], F16, kind="ExternalInput").ap()
    outs = [
        nc.dram_tensor(f"out{h}", [BPC, P, NCH, D], F16, kind="ExternalOutput").ap()
        for h in range(2)
    ]

    Exp = mybir.ActivationFunctionType.Exp
    Copy = mybir.ActivationFunctionType.Copy

    with tile.TileContext(nc) as tc:
        with (
            tc.tile_pool(name="consts", bufs=1) as consts,
            tc.tile_pool(name="batch", bufs=2) as batch_pool,
            tc.tile_pool(name="proj", bufs=2) as proj_pool,
            tc.tile_pool(name="work", bufs=2) as work,
            tc.tile_pool(name="outp", bufs=2) as outp,
            tc.tile_pool(name="stats", bufs=8) as stats,
            tc.tile_pool(name="pa", bufs=1, space="PSUM") as pa,    # 1 tag x 2 banks
            tc.tile_pool(name="pd", bufs=2, space="PSUM") as pd,    # 1 tag x 2 banks
            tc.tile_pool(name="pf", bufs=1, space="PSUM") as pf,    # 2 tags x 1 bank
        ):
            # PE warmup scratch first (gpsimd boots early)
            wz = consts.tile([P, TS], BF16, tag="wz")
            nc.gpsimd.memset(wz[:], 0.0)
            nbias = consts.tile([P, 1], F32, tag="nbias")
            nc.vector.memset(nbias[:], -CBIAS)

            wu = pa.tile([P, 2, Wn], F32, name="wu", tag="pam")
            for i in range(NWU):
                nc.tensor.matmul(
                    wu[:, 0, :], wz[:, :P], wz[:], start=(i == 0), stop=(i == NWU - 1)
                )

            # projection weights, transposed: wt_sb[h][k % 128, kc, d] = W_h[d, k]
            # (host bakes in: chunk2 = rows 256:300 at partitions 0:44, chunk3 =
            # the same rows duplicated at partitions 64:108, zeros elsewhere)
            wt_sb = []
            for h in range(2):
                t_ = consts.tile([P, 4, D], F16, tag=f"wt{h}")
                nc.sync.dma_start(t_[:], wTr[:, h, :, :])
                wt_sb.append(t_)

            # --- input loads for ALL batches up front, spread across queues,
            # critical-path first; later transfers overlap compute ---
            HT = T // 2
            ats, mains, m44s, afs = [], [], [], []
            for b in range(BPC):
                at_sb = batch_pool.tile([P, 4, Wn], F16, tag="attnT")
                nc.scalar.dma_start(at_sb[:, 0:2, :], attnTr[b, :, 0:2, :])
                # batch 0's attn chunks ride the fast path; later batches'
                # go via the otherwise-idle scalar queue so the scheduler's
                # hoisted A-phase matmuls for them never stall the PE FIFO
                # and batch 0's gpsimd loads are not delayed
                if b == 0:
                    nc.gpsimd.dma_start(at_sb[:, 2:4, :], attnTr[b, :, 2:4, :])
                else:
                    nc.scalar.dma_start(at_sb[:, 2:4, :], attnTr[b, :, 2:4, :])
                main_sb = batch_pool.tile([P, 2, T], F16, tag="main")
                nc.gpsimd.dma_start(main_sb[:, 0, 0:HT], mainT[b, 0:P, 0:HT])
                nc.sync.dma_start(main_sb[:, 1, 0:HT], mainT[b, P : 2 * P, 0:HT])
                m44b = batch_pool.tile([P, T], F16, tag="m44")
                nc.gpsimd.dma_start(m44b[:K44, :], main44[b])
                nc.gpsimd.dma_start(m44b[64 : 64 + K44, :], main44[b])
                af_sb = batch_pool.tile([P, WCH, D + 2], BF16, tag="attnF")
                nc.gpsimd.dma_start(af_sb[:], attnF[b])
                nc.gpsimd.dma_start(main_sb[:, 0, HT:T], mainT[b, 0:P, HT:T])
                nc.sync.dma_start(main_sb[:, 1, HT:T], mainT[b, P : 2 * P, HT:T])
                ats.append(at_sb)
                mains.append(main_sb)
                m44s.append(m44b)
                afs.append(af_sb)

            for b in range(BPC):
                at_sb, main_sb, m44b, af_sb = ats[b], mains[b], m44s[b], afs[b]

                # --- A: projections for BOTH heads up front.  h=0 uses the pa
                # banks, h=1 borrows a pd ring slot (D hasn't started yet) so
                # h=1's matmuls never wait on h=0's PSUM evacuation. ---
                projTs, projT44s = [], []
                for h in range(2):
                    projT = proj_pool.tile([P, 2, Wn], F16, tag=f"projT{h}")
                    projT44b = proj_pool.tile([P, Wn], F16, tag=f"projT44{h}")
                    if h == 0:
                        pam = pa.tile([P, 2, Wn], F32, name="pam", tag="pam")
                    else:
                        pam = pd.tile([P, 2, Wn], F32, name="pam_d", tag="ps_dm")
                    for kc in (0, 1):
                        for j in range(2):
                            nc.tensor.matmul(
                                pam[:, j, :],
                                wt_sb[h][:P, kc, j * P : (j + 1) * P],
                                at_sb[:P, kc, :],
                                start=(kc == 0),
                                stop=False,
                            )
                    # k44 tail, row-packed: j=0 in rows 0:64, j=1 in rows 64:128
                    nc.tensor.matmul(
                        pam[:, 0, :],
                        wt_sb[h][:K44, 2, 0:P],
                        at_sb[:K44, 2, :],
                        start=False,
                        stop=True,
                    )
                    nc.tensor.matmul(
                        pam[:, 1, :],
                        wt_sb[h][64 : 64 + K44, 3, P : 2 * P],
                        at_sb[64 : 64 + K44, 3, :],
                        start=False,
                        stop=True,
                    )
                    nc.scalar.activation(projT[:], pam[:], Copy)
                    # proj rows 256:300 -> fp16 straight from PSUM (pf bank is
                    # idle during the A phase)
                    pa2 = pf.tile([P, Wn], F32, name="ps_a2", tag=f"ps_f{h}")
                    for kc in (0, 1):
                        nc.tensor.matmul(
                            pa2[:K44, :],
                            wt_sb[h][:P, kc, 2 * P : D],
                            at_sb[:P, kc, :],
                            start=(kc == 0),
                            stop=False,
                        )
                    nc.tensor.matmul(
                        pa2[:K44, :],
                        wt_sb[h][:K44, 2, 2 * P : D],
                        at_sb[:K44, 2, :],
                        start=False,
                        stop=True,
                    )
                    nc.scalar.activation(projT44b[:K44, :], pa2[:K44, :], Copy)
                    # duplicate at partitions 64:108 for row-packed D tails
                    nc.scalar.dma_start(projT44b[64 : 64 + K44, :], projT44b[:K44, :])
                    projTs.append(projT)
                    projT44s.append(projT44b)

                for h in range(2):
                    projT, projT44b = projTs[h], projT44s[h]
                    o_big = outp.tile([P, NCH, D], F16, tag=f"o{h}")
                    for t5 in range(TSN):
                        ts0 = t5 * TS
                        # D: scoresT[w, t] for a wc-pair into a 2-bank PSUM
                        # tile; packed k44 pair leads, then fp32r chunks.
                        es = work.tile([P, WCH, TS], BF16, tag="es")
                        for wp in range(WCH // 2):
                            wcs = (2 * wp, 2 * wp + 1)
                            pdm = pd.tile([P, 2, TS], F32, name="ps_dm", tag="ps_dm")
                            for kc in (0, 1):
                                for j, wc in enumerate(wcs):
                                    nc.tensor.matmul(
                                        pdm[:, j, :],
                                        projT[:P, kc, wc * P : (wc + 1) * P],
                                        main_sb[:P, kc, ts0 : ts0 + TS],
                                        start=(kc == 0),
                                        stop=False,
                                    )
                            # k44 tails, row-packed -> concurrent pair
                            nc.tensor.matmul(
                                pdm[:, 0, :],
                                projT44b[:K44, wcs[0] * P : (wcs[0] + 1) * P],
                                m44b[:K44, ts0 : ts0 + TS],
                                start=False,
                                stop=True,
                            )
                            nc.tensor.matmul(
                                pdm[:, 1, :],
                                projT44b[64 : 64 + K44, wcs[1] * P : (wcs[1] + 1) * P],
                                m44b[64 : 64 + K44, ts0 : ts0 + TS],
                                start=False,
                                stop=True,
                            )
                            nc.scalar.activation(
                                es[:, 2 * wp : 2 * wp + 2, :],
                                pdm[:],
                                Exp,
                                bias=nbias[:],
                                scale=1.0,
                            )
                        # F: [out | Z] = es.T @ [attn | 1]; out /= Z
                        for tp in range(TS // P // 2):
                            tcs = (2 * tp * P, (2 * tp + 1) * P)
                            pfs = [
                                pf.tile([P, Wn], F32, name=f"ps_f{j}", tag=f"ps_f{j}")
                                for j in range(2)
                            ]
                            for wc in range(WCH):
                                for j, tc0 in enumerate(tcs):
                                    nc.tensor.matmul(
                                        pfs[j][:, : D + 2],
                                        es[:, wc, tc0 : tc0 + P],
                                        af_sb[:, wc, :],
                                        start=(wc == 0),
                                        stop=(wc == WCH - 1),
                                    )
                            last = b == BPC - 1 and h == 1 and t5 == TSN - 1
                            for j, tc0 in enumerate(tcs):
                                c = t5 * (TS // P) + tc0 // P
                                rz = stats.tile([P, 1], F32, tag="rz")
                                nc.vector.reciprocal(rz[:], pfs[j][:, D : D + 1])
                                if last and tp == 0:
                                    # final slab: split divides across ACT and
                                    # DVE so they finish concurrently
                                    nc.scalar.activation(
                                        o_big[:, c, :], pfs[j][:, :D], Copy, scale=rz[:]
                                    )
                                else:
                                    nc.vector.tensor_scalar_mul(
                                        o_big[:, c, :], pfs[j][:, :D], rz[:]
                                    )
                            if last:
                                for j, tc0 in enumerate(tcs):
                                    c = t5 * (TS // P) + tc0 // P
                                    nc.sync.dma_start(
                                        outs[h][b, :, c : c + 1, :], o_big[:, c : c + 1, :]
                                    )
                        if not (b == BPC - 1 and h == 1 and t5 == TSN - 1):
                            c0 = t5 * (TS // P)
                            nc.sync.dma_start(
                                outs[h][b, :, c0 : c0 + 4, :], o_big[:, c0 : c0 + 4, :]
                            )

    nc.compile()
    return nc


def _get_program():
    global _cached
    if _cached is None:
        _cached = _build_program()
    return _cached


def _prep_in_maps(input1, input2, W2, W3):
    input1 = np.ascontiguousarray(input1, dtype=np.float32)
    input2 = np.ascontiguousarray(input2, dtype=np.float32)
    wt = np.stack([np.asarray(W2).T, np.asarray(W3).T]).astype(np.float32)  # [2, D, D]
    wtr = np.zeros((P, 2, 4, D), np.float32)
    for kc in (0, 1):
        wtr[:, :, kc, :] = wt[:, kc * P : (kc + 1) * P, :].transpose(1, 0, 2)
    wtr[:K44, :, 2, :] = wt[:, 2 * P : D, :].transpose(1, 0, 2)
    wtr[64 : 64 + K44, :, 3, :] = wt[:, 2 * P : D, :].transpose(1, 0, 2)
    wtr = np.ascontiguousarray(wtr).astype(np.float16)
    in_maps = []
    for c in range(NCORES):
        sl = slice(c * BPC, (c + 1) * BPC)
        i1t = input1[sl].transpose(0, 2, 1)  # [BPC, D, T]
        i2 = input2[sl]
        i2t = i2.transpose(0, 2, 1)  # [BPC, D, Wn]
        atr = np.zeros((BPC, P, 4, Wn), np.float32)
        for kc in (0, 1):
            atr[:, :, kc, :] = i2t[:, kc * P : (kc + 1) * P, :]
        atr[:, :K44, 2, :] = i2t[:, 2 * P : D, :]
        atr[:, 64 : 64 + K44, 3, :] = i2t[:, 2 * P : D, :]
        af = np.ones((BPC, WCH, P, D + 2), np.float32)
        af[:, :, :, :D] = i2.reshape(BPC, WCH, P, D)
        in_maps.append(
            {
                "mainT": np.ascontiguousarray(i1t[:, :256, :]).astype(np.float16),
                "attnTr": np.ascontiguousarray(atr).astype(np.float16),
                "attnF": np.ascontiguousarray(af.transpose(0, 2, 1, 3)).astype(ml_dtypes.bfloat16),
                "wTr": wtr,
                "main44": np.ascontiguousarray(i1t[:, 256:D, :]).astype(np.float16),
            }
        )
    return in_maps


def kernel(input1, input2, W2, b2, W3, b3, mode, _trace=False):
    mode = int(np.asarray(mode))
    if mode not in (0, 1):
        raise AttributeError("Wrong mode!")

    nc = _get_program()
    in_maps = _prep_in_maps(input1, input2, W2, W3)
    res = bass_utils.run_bass_kernel_spmd(
        nc, in_maps, core_ids=list(range(NCORES)), trace=_trace
    )
    full_outs = []
    for h in range(2):
        parts = [np.asarray(r[f"out{h}"]) for r in res.results]  # [BPC, P, NCH, D] f16
        full = np.concatenate(parts, axis=0)  # [B, P, NCH, D]
        full = (
            full.transpose(0, 2, 1, 3).reshape(B, T, D).astype(np.float32)
        )  # t = c*128 + p
        full_outs.append(full)
    if _trace:
        kernel.last_results = res
    if mode == 0:
        return full_outs[0]
    return (full_outs[0], full_outs[1])


# revision 19
# speedup vs baseline: 1.1652x; 1.0103x over previous
"""BiAttention Trainium2 kernel (nn_BiAttention_76794015252634).

reference math (mode=1), per batch b:
    proj_h = attn @ Wh.T + bh          # [Wn, D]
    scores = main @ proj_h.T           # [T, Wn]
    probs  = softmax(scores, axis=-1)
    out_h  = probs @ attn              # [T, D]
for h in {2, 3}; returns (out_2, out_3).

Design notes (v3):
  * bias bh cancels in softmax (row-constant) -> skipped.
  * softmax shift: fixed constant C=100 instead of per-row max; scores are
    built transposed (w-major) so no transposes of the probabilities.
  * denominator Z falls out of the context matmul via a ones-column in attn.
  * D (scores) stationary operand projT is fp16: its LDWEIGHTS gets fast
    weight load and hides fully under the previous matmul's 213ns fill,
    unlike fp32's 190ns exposed load. The moving operand stays fp32r.
  * the 44-row tail of the D=300 contraction is fp16 and ROW-PACKED via PE
    tile rows 0:64 / 64:128 so the two tail matmuls of a wc-pair run
    concurrently; the pair leads the accumulation group.
  * scoresT slabs for a wc-pair land in one 2-bank PSUM tile and leave via a
    single 1024-wide exp; one of the 4 per-slab divides runs on the scalar
    engine so neither ACT nor DVE exceeds the PE's per-slab time.
  * outputs accumulate per (batch, head) in SBUF fp16 and leave per-slab in
    per-partition-contiguous DMAs on the sync queue; host undoes the
    [P, chunk] interleave and upcasts.
  * inputs arrive as consolidated per-partition-contiguous images (weights
    1 DMA/head, attn 1 DMA/batch with the k44 dup + zero pad baked in on
    host) spread across 4 engine queues; dummy warmup matmuls keep the PE
    HAM clock-gate warming while they land.

Per (batch, head):
    A: projT[d, w]   = sum_k WhT[k, d] attnT[k, w]          (PE, PSUM->SBUF)
    D: scoresT[w, t] = sum_d projT[d, w] mainT[d, t]        (PE)
       es[w, t]      = exp(scoresT - C)                     (ACT, PSUM->SBUF)
    F: [out | Z][t]  = sum_w es[w, t] [attn | 1][w, :]      (PE)
       out[t, d]     = out[t, d] / Z[t]                     (DVE recip + mul)

Sharding: data-parallel over batch, B=16 -> 2 batches per core on 8 cores.
"""

import ml_dtypes
import numpy as np

import concourse.bass as bass
import concourse.tile as tile
from concourse import bacc, mybir
from concourse import bass_utils

B, T, Wn, D = 16, 2048, 512, 300
NCORES = 8
BPC = B // NCORES  # batches per core
P = 128
WCH = Wn // P      # 4 w-chunks
TS = 512           # t slab width (one PSUM bank)
TSN = T // TS      # 4 slabs
NCH = T // P       # 16 output t-chunks per (b,h)
K44 = 44           # tail rows of the D=300 contraction
CBIAS = 100.0      # softmax shift constant (see module docstring)
NWU = 8            # warmup matmuls

F32 = mybir.dt.float32
F32R = mybir.dt.float32r
BF16 = mybir.dt.bfloat16
F16 = mybir.dt.float16

_cached = None


def _build_program():
    nc = bacc.Bacc("TRN2", target_bir_lowering=False, debug=False)

    mainT = nc.dram_tensor("mainT", [BPC, 256, T], F16, kind="ExternalInput").ap()
    attnTr = nc.dram_tensor("attnTr", [BPC, P, 4, Wn], F16, kind="ExternalInput").ap()
    attnF = nc.dram_tensor("attnF", [BPC, P, WCH, D + 2], BF16, kind="ExternalInput").ap()
    wTr = nc.dram_tensor("wTr", [P, 2, 4, D], F16, kind="ExternalInput").ap()
    main44 = nc.dram_tensor("main44", [BPC, K44, T], F16, kind="ExternalInput").ap()
    outs = [
        nc.dram_tensor(f"out{h}", [BPC, P, NCH, D], F16, kind="ExternalOutput").ap()
        for h in range(2)
    ]

    Exp = mybir.ActivationFunctionType.Exp
    Copy = mybir.ActivationFunctionType.Copy

    with tile.TileContext(nc) as tc:
        with (
            tc.tile_pool(name="consts", bufs=1) as consts,
            tc.tile_pool(name="batch", bufs=2) as batch_pool,
            tc.tile_pool(name="proj", bufs=2) as proj_pool,
            tc.tile_pool(name="work", bufs=2) as work,
            tc.tile_pool(name="outp", bufs=2) as outp,
            tc.tile_pool(name="stats", bufs=8) as stats,
            tc.tile_pool(name="pd", bufs=3, space="PSUM") as pd,    # 1 tag x 2 banks
            tc.tile_pool(name="pf", bufs=1, space="PSUM") as pf,    # 2 tags x 1 bank
        ):
            # PE warmup scratch first (gpsimd boots early)
            wz = consts.tile([P, TS], BF16, tag="wz")
            nc.gpsimd.memset(wz[:], 0.0)
            nbias = consts.tile([P, 1], F32, tag="nbias")
            nc.vector.memset(nbias[:], -CBIAS)

            wu = pd.tile([P, 2, Wn], F32, name="wu", tag="ps_dm")
            for i in range(NWU):
                nc.tensor.matmul(
                    wu[:, 0, :], wz[:, :P], wz[:], start=(i == 0), stop=(i == NWU - 1)
                )

            # projection weights, transposed: wt_sb[h][k % 128, kc, d] = W_h[d, k]
            # (host bakes in: chunk2 = rows 256:300 at partitions 0:44, chunk3 =
            # the same rows duplicated at partitions 64:108, zeros elsewhere)
            wt_sb = []
            for h in range(2):
                t_ = consts.tile([P, 4, D], F16, tag=f"wt{h}")
                nc.sync.dma_start(t_[:], wTr[:, h, :, :])
                wt_sb.append(t_)

            # --- input loads for ALL batches up front, spread across queues,
            # critical-path first; later transfers overlap compute ---
            HT = T // 2
            ats, mains, m44s, afs = [], [], [], []
            for b in range(BPC):
                at_sb = batch_pool.tile([P, 4, Wn], F16, tag="attnT")
                nc.scalar.dma_start(at_sb[:, 0:2, :], attnTr[b, :, 0:2, :])
                # batch 0's attn chunks ride the fast path; later batches'
                # go via the otherwise-idle scalar queue so the scheduler's
                # hoisted A-phase matmuls for them never stall the PE FIFO
                # and batch 0's gpsimd loads are not delayed
                if b == 0:
                    nc.gpsimd.dma_start(at_sb[:, 2:4, :], attnTr[b, :, 2:4, :])
                else:
                    nc.scalar.dma_start(at_sb[:, 2:4, :], attnTr[b, :, 2:4, :])
                main_sb = batch_pool.tile([P, 2, T], F16, tag="main")
                nc.gpsimd.dma_start(main_sb[:, 0, 0:HT], mainT[b, 0:P, 0:HT])
                nc.sync.dma_start(main_sb[:, 1, 0:HT], mainT[b, P : 2 * P, 0:HT])
                m44b = batch_pool.tile([P, T], F16, tag="m44")
                nc.gpsimd.dma_start(m44b[:K44, :], main44[b])
                nc.gpsimd.dma_start(m44b[64 : 64 + K44, :], main44[b])
                af_sb = batch_pool.tile([P, WCH, D + 2], BF16, tag="attnF")
                nc.gpsimd.dma_start(af_sb[:], attnF[b])
                nc.gpsimd.dma_start(main_sb[:, 0, HT:T], mainT[b, 0:P, HT:T])
                nc.sync.dma_start(main_sb[:, 1, HT:T], mainT[b, P : 2 * P, HT:T])
                ats.append(at_sb)
                mains.append(main_sb)
                m44s.append(m44b)
                afs.append(af_sb)

            for b in range(BPC):
                at_sb, main_sb, m44b, af_sb = ats[b], mains[b], m44s[b], afs[b]

                # --- A: projections for BOTH heads up front.  h=0 uses the pa
                # banks, h=1 borrows a pd ring slot (D hasn't started yet) so
                # h=1's matmuls never wait on h=0's PSUM evacuation. ---
                projTs, projT44s = [], []
                for h in range(2):
                    projT = proj_pool.tile([P, 2, Wn], F16, tag=f"projT{h}")
                    projT44b = proj_pool.tile([P, Wn], F16, tag=f"projT44{h}")
                    pam = pd.tile([P, 2, Wn], F32, name="pam", tag="ps_dm")
                    for kc in (0, 1):
                        for j in range(2):
                            nc.tensor.matmul(
                                pam[:, j, :],
                                wt_sb[h][:P, kc, j * P : (j + 1) * P],
                                at_sb[:P, kc, :],
                                start=(kc == 0),
                                stop=False,
                            )
                    # k44 tail, row-packed: j=0 in rows 0:64, j=1 in rows 64:128
                    nc.tensor.matmul(
                        pam[:, 0, :],
                        wt_sb[h][:K44, 2, 0:P],
                        at_sb[:K44, 2, :],
                        start=False,
                        stop=True,
                    )
                    nc.tensor.matmul(
                        pam[:, 1, :],
                        wt_sb[h][64 : 64 + K44, 3, P : 2 * P],
                        at_sb[64 : 64 + K44, 3, :],
                        start=False,
                        stop=True,
                    )
                    nc.scalar.activation(projT[:], pam[:], Copy)
                    # proj rows 256:300 -> fp16 straight from PSUM (pf bank is
                    # idle during the A phase)
                    pa2 = pf.tile([P, Wn], F32, name="ps_a2", tag=f"ps_f{h}")
                    for kc in (0, 1):
                        nc.tensor.matmul(
                            pa2[:K44, :],
                            wt_sb[h][:P, kc, 2 * P : D],
                            at_sb[:P, kc, :],
                            start=(kc == 0),
                            stop=False,
                        )
                    nc.tensor.matmul(
                        pa2[:K44, :],
                        wt_sb[h][:K44, 2, 2 * P : D],
                        at_sb[:K44, 2, :],
                        start=False,
                        stop=True,
                    )
                    nc.scalar.activation(projT44b[:K44, :], pa2[:K44, :], Copy)
                    # duplicate at partitions 64:108 for row-packed D tails
                    nc.scalar.dma_start(projT44b[64 : 64 + K44, :], projT44b[:K44, :])
                    projTs.append(projT)
                    projT44s.append(projT44b)

                for h in range(2):
                    projT, projT44b = projTs[h], projT44s[h]
                    o_big = outp.tile([P, NCH, D], F16, tag=f"o{h}")
                    for t5 in range(TSN):
                        ts0 = t5 * TS
                        # D: scoresT[w, t] for a wc-pair into a 2-bank PSUM
                        # tile; packed k44 pair leads, then fp32r chunks.
                        es = work.tile([P, WCH, TS], BF16, tag="es")
                        for wp in range(WCH // 2):
                            wcs = (2 * wp, 2 * wp + 1)
                            pdm = pd.tile([P, 2, TS], F32, name="ps_dm", tag="ps_dm")
                            for kc in (0, 1):
                                for j, wc in enumerate(wcs):
                                    nc.tensor.matmul(
                                        pdm[:, j, :],
                                        projT[:P, kc, wc * P : (wc + 1) * P],
                                        main_sb[:P, kc, ts0 : ts0 + TS],
                                        start=(kc == 0),
                                        stop=False,
                                    )
                            # k44 tails, row-packed -> concurrent pair
                            nc.tensor.matmul(
                                pdm[:, 0, :],
                                projT44b[:K44, wcs[0] * P : (wcs[0] + 1) * P],
                                m44b[:K44, ts0 : ts0 + TS],
                                start=False,
                                stop=True,
                            )
                            nc.tensor.matmul(
                                pdm[:, 1, :],
                                projT44b[64 : 64 + K44, wcs[1] * P : (wcs[1] + 1) * P],
                                m44b[64 : 64 + K44, ts0 : ts0 + TS],
                                start=False,
                                stop=True,
                            )
                            nc.scalar.activation(
                                es[:, 2 * wp : 2 * wp + 2, :],
                                pdm[:],
                                Exp,
                                bias=nbias[:],
                                scale=1.0,
                            )
                        # F: [out | Z] = es.T @ [attn | 1]; out /= Z
                        for tp in range(TS // P // 2):
                            tcs = (2 * tp * P, (2 * tp + 1) * P)
                            pfs = [
                                pf.tile([P, Wn], F32, name=f"ps_f{j}", tag=f"ps_f{j}")
                                for j in range(2)
                            ]
                            for wc in range(WCH):
                                for j, tc0 in enumerate(tcs):
                                    nc.tensor.matmul(
                                        pfs[j][:, : D + 2],
                                        es[:, wc, tc0 : tc0 + P],
                                        af_sb[:, wc, :],
                                        start=(wc == 0),
                                        stop=(wc == WCH - 1),
                                    )
                            last = b == BPC - 1 and h == 1 and t5 == TSN - 1
                            for j, tc0 in enumerate(tcs):
                                c = t5 * (TS // P) + tc0 // P
                                rz = stats.tile([P, 1], F32, tag="rz")
                                nc.vector.reciprocal(rz[:], pfs[j][:, D : D + 1])
                                if last and tp == 0:
                                    # final slab: split divides across ACT and
                                    # DVE so they finish concurrently
                                    nc.scalar.activation(
                                        o_big[:, c, :], pfs[j][:, :D], Copy, scale=rz[:]
                                    )
                                else:
                                    nc.vector.tensor_scalar_mul(
                                        o_big[:, c, :], pfs[j][:, :D], rz[:]
                                    )
                            if last:
                                for j, tc0 in enumerate(tcs):
                                    c = t5 * (TS // P) + tc0 // P
                                    nc.sync.dma_start(
                                        outs[h][b, :, c : c + 1, :], o_big[:, c : c + 1, :]
                                    )
                        if not (b == BPC - 1 and h == 1 and t5 == TSN - 1):
                            c0 = t5 * (TS // P)
                            nc.sync.dma_start(
                                outs[h][b, :, c0 : c0 + 4, :], o_big[:, c0 : c0 + 4, :]
                            )

    nc.compile()
    return nc


def _get_program():
    global _cached
    if _cached is None:
        _cached = _build_program()
    return _cached


def _prep_in_maps(input1, input2, W2, W3):
    input1 = np.ascontiguousarray(input1, dtype=np.float32)
    input2 = np.ascontiguousarray(input2, dtype=np.float32)
    wt = np.stack([np.asarray(W2).T, np.asarray(W3).T]).astype(np.float32)  # [2, D, D]
    wtr = np.zeros((P, 2, 4, D), np.float32)
    for kc in (0, 1):
        wtr[:, :, kc, :] = wt[:, kc * P : (kc + 1) * P, :].transpose(1, 0, 2)
    wtr[:K44, :, 2, :] = wt[:, 2 * P : D, :].transpose(1, 0, 2)
    wtr[64 : 64 + K44, :, 3, :] = wt[:, 2 * P : D, :].transpose(1, 0, 2)
    wtr = np.ascontiguousarray(wtr).astype(np.float16)
    in_maps = []
    for c in range(NCORES):
        sl = slice(c * BPC, (c + 1) * BPC)
        i1t = input1[sl].transpose(0, 2, 1)  # [BPC, D, T]
        i2 = input2[sl]
        i2t = i2.transpose(0, 2, 1)  # [BPC, D, Wn]
        atr = np.zeros((BPC, P, 4, Wn), np.float32)
        for kc in (0, 1):
            atr[:, :, kc, :] = i2t[:, kc * P : (kc + 1) * P, :]
        atr[:, :K44, 2, :] = i2t[:, 2 * P : D, :]
        atr[:, 64 : 64 + K44, 3, :] = i2t[:, 2 * P : D, :]
        af = np.ones((BPC, WCH, P, D + 2), np.float32)
        af[:, :, :, :D] = i2.reshape(BPC, WCH, P, D)
        in_maps.append(
            {
                "mainT": np.ascontiguousarray(i1t[:, :256, :]).astype(np.float16),
                "attnTr": np.ascontiguousarray(atr).astype(np.float16),
                "attnF": np.ascontiguousarray(af.transpose(0, 2, 1, 3)).astype(ml_dtypes.bfloat16),
                "wTr": wtr,
                "main44": np.ascontiguousarray(i1t[:, 256:D, :]).astype(np.float16),
            }
        )
    return in_maps


def kernel(input1, input2, W2, b2, W3, b3, mode, _trace=False):
    mode = int(np.asarray(mode))
    if mode not in (0, 1):
        raise AttributeError("Wrong mode!")

    nc = _get_program()
    in_maps = _prep_in_maps(input1, input2, W2, W3)
    res = bass_utils.run_bass_kernel_spmd(
        nc, in_maps, core_ids=list(range(NCORES)), trace=_trace
    )
    full_outs = []
    for h in range(2):
        parts = [np.asarray(r[f"out{h}"]) for r in res.results]  # [BPC, P, NCH, D] f16
        full = np.concatenate(parts, axis=0)  # [B, P, NCH, D]
        full = (
            full.transpose(0, 2, 1, 3).reshape(B, T, D).astype(np.float32)
        )  # t = c*128 + p
        full_outs.append(full)
    if _trace:
        kernel.last_results = res
    if mode == 0:
        return full_outs[0]
    return (full_outs[0], full_outs[1])
